# revision 45
# baseline (speedup 1.0000x reference)
"""Multi-head attention (B=2, S=4096, D=512, H=8) on 8 TRN2 NeuronCores.

Sharding: batch x head-pair. Core c handles batch b=c//4, head pair
p=c%4 (heads 2p, 2p+1 = model dims 128p..128p+128), and ALL 4096 query
tokens of its batch. Each core projects only its own 128-dim slice of
Q^/K^/V^ (no duplicated projection work across the batch group), runs
flash-style attention for its 2 heads over the full 4096x4096 score
block, and emits the PARTIAL output projection O_pair^T @ W_o[pair, :]
in bf16 for all 4096 tokens. The host sums the 4 partials per batch
and adds the effective bias (b_o + b_v @ W_o^T), so no cross-core
communication is needed on device.

All matmuls in bf16 with fp32 PSUM accumulation. Softmax skips the
max-subtraction (scores ~ N(0,1); exp is safe in fp32) and the
denominator comes from a ones column appended to V^, so softmax costs
exactly one ACT pass per score tile. Q/K biases are folded into the
PSUM->SBUF copy on the vector engine (zero bias matmuls).

Pipelining: scores for unit u+1 are emitted before PV of unit u and
ahead of any interleaved projection work, so ACT (the pacing engine in
steady state, ~1.1us per 128x1024 exp) never waits; K/Q/V projections,
normalization chains, and output-projection tiles ride the unit
stream. The last block's second head skips normalization: its
out-projection runs unnormalized and the reciprocal - produced by
transposing the denominator row into PSUM columns with K=1 matmuls
(no DMA round trip) - is applied per-partition in the final
scalar_tensor_tensor. The tensor engine is pre-warmed on dummy
matmuls during the prologue DMA window (p-state ramp).
"""

import numpy as np
import ml_dtypes

B, S, D = 2, 4096, 512
H, DK = 8, 64
N_CORES = 8
HP = 2  # heads per core (one pair)
NT = 4  # tq blocks of 1024
TQB = 1024

_PROGRAM = None


def _build_program():
    from contextlib import ExitStack

    import concourse.mybir as mybir
    import concourse.tile as tile
    from concourse import bacc

    bf = mybir.dt.bfloat16
    f32 = mybir.dt.float32
    Exp = mybir.ActivationFunctionType.Exp
    Add = mybir.AluOpType.add
    Mult = mybir.AluOpType.mult
    Bypass = mybir.AluOpType.bypass

    nc = bacc.Bacc(None)

    qT = nc.declare_dram_parameter("qT", [D, S], bf, isOutput=False)
    kT = nc.declare_dram_parameter("kT", [D, S], bf, isOutput=False)
    vT = nc.declare_dram_parameter("vT", [D, S], bf, isOutput=False)
    wqPT = nc.declare_dram_parameter("wqPT", [D, 128], bf, isOutput=False)
    wkPT = nc.declare_dram_parameter("wkPT", [D, 128], bf, isOutput=False)
    wvPT = nc.declare_dram_parameter("wvPT", [D, 128], bf, isOutput=False)
    woP = nc.declare_dram_parameter("woP", [128, D], bf, isOutput=False)
    bqPc = nc.declare_dram_parameter("bqPc", [128, 1], f32, isOutput=False)
    bkPc = nc.declare_dram_parameter("bkPc", [128, 1], f32, isOutput=False)
    out_p = nc.declare_dram_parameter("out", [S, D], bf, isOutput=True)
    # DRAM scratch rows for softmax denominator / reciprocal broadcasting
    rden = nc.dram_tensor("rden", [HP * NT * 2, 512], bf)
    rrec = nc.dram_tensor("rrec", [HP * NT * 2, 512], f32)

    with tile.TileContext(nc) as tc, ExitStack() as ctx:
        wpool = ctx.enter_context(tc.tile_pool(name="w", bufs=1))
        qstream = ctx.enter_context(tc.tile_pool(name="qstream", bufs=2))
        kstream = ctx.enter_context(tc.tile_pool(name="kstream", bufs=2))
        vstream = ctx.enter_context(tc.tile_pool(name="vstream", bufs=2))
        qtres = ctx.enter_context(tc.tile_pool(name="qtres", bufs=1))
        ktres = ctx.enter_context(tc.tile_pool(name="ktres", bufs=1))
        vstore = ctx.enter_context(tc.tile_pool(name="vstore", bufs=32))
        ppool = ctx.enter_context(tc.tile_pool(name="p", bufs=4))
        opool = ctx.enter_context(tc.tile_pool(name="o", bufs=1))
        wspool = ctx.enter_context(tc.tile_pool(name="ws", bufs=4))
        ostage = ctx.enter_context(tc.tile_pool(name="ostage", bufs=3))
        pasb = ctx.enter_context(tc.tile_pool(name="pasb", bufs=8))
        projp = ctx.enter_context(tc.tile_pool(name="projp", bufs=2, space="PSUM"))
        scorep = ctx.enter_context(tc.tile_pool(name="scorep", bufs=2, space="PSUM"))
        pvp = ctx.enter_context(tc.tile_pool(name="pvp", bufs=2, space="PSUM"))

        dma = nc.sync.dma_start
        MM = nc.tensor.matmul

        def wtile(param, tagp):
            t = wpool.tile([128, 4, 128], bf, tag=tagp, name=tagp)
            dma(out=t[:], in_=param[:].rearrange("(c p) m -> p c m", p=128))
            return t

        # ---- essential constants first (everything else is deferred) ----
        wq_t = wtile(wqPT, "wq")
        bq_t = wpool.tile([128, 1], f32, tag="bq", name="bq_t")
        dma(out=bq_t[:], in_=bqPc[:])
        wk_t = wtile(wkPT, "wk")
        bk_t = wpool.tile([128, 1], f32, tag="bk", name="bk_t")
        dma(out=bk_t[:], in_=bkPc[:])
        # dummy SBUF operand for scalar_tensor_tensor bias adds (op1=bypass
        # ignores it, but only one non-scalar input may come from PSUM)
        dummy = wpool.tile([128, 512], bf, tag="dummy", name="dummy")
        nc.vector.memset(dummy[:], 0.0)
        ones_bf = wpool.tile([128, 1], bf, tag="ones1", name="ones_bf")
        nc.vector.memset(ones_bf[:], 1.0)

        qt = qtres.tile([128, S], bf, tag="qt", name="qt")
        kt = ktres.tile([128, S], bf, tag="kt", name="kt")
        o_pair = opool.tile([128, S], bf, tag="o", name="o_pair")
        v_store = []  # 32 tiles [128, HP, DK+1]; last col per head = ones

        def load_raw(src, c, eng=None):
            raw = (qstream if src is qT else kstream).tile(
                [128, 4, 512], bf, tag="raw", name="raw"
            )
            (eng.dma_start if eng is not None else dma)(
                out=raw[:],
                in_=src[:, c * 512 : (c + 1) * 512].rearrange(
                    "(c p) t -> p c t", p=128
                ),
            )
            return raw

        def proj_from_raw(raw, w_t, b_t, dst, c):
            ps = projp.tile([128, 512], f32, tag="proj", name="proj_ps")
            for kk in range(4):
                MM(
                    ps[:],
                    w_t[:, kk, :],
                    raw[:, kk, :],
                    start=(kk == 0),
                    stop=(kk == 3),
                    skip_group_check=True,
                )
            nc.vector.scalar_tensor_tensor(
                out=dst[:, c * 512 : (c + 1) * 512],
                in0=ps[:],
                scalar=b_t[:],
                in1=dummy[:],
                op0=Add,
                op1=Bypass,
            )

        def proj_chunk(src, w_t, b_t, dst, c):
            proj_from_raw(load_raw(src, c), w_t, b_t, dst, c)

        # ---- deferred constant loads / computations (closures) ----
        wv_t = None
        vraw_tiles = {}

        def load_wv():
            nonlocal wv_t
            wv_t = wtile(wvPT, "wv")

        def load_vraw(c, eng=None):
            t = vstream.tile([128, 4, 512], bf, tag="vraw", name="vraw")
            (eng.dma_start if eng is not None else dma)(
                out=t[:],
                in_=vT[:, c * 512 : (c + 1) * 512].rearrange(
                    "(c p) t -> p c t", p=128
                ),
            )
            vraw_tiles[c] = t

        wo_t = wo_lo = None

        def load_wo():
            nonlocal wo_t, wo_lo
            wo_t = wpool.tile([128, D], bf, tag="wo", name="wo_t")
            dma(out=wo_t[:], in_=woP[:])
            # head-1 rows staged at base partition 0 so the tail pb matmuls
            # can pair them with pvsb tiles (also at partition 0)
            wo_lo = wpool.tile([64, D], bf, tag="wolo", name="wo_lo")
            dma(out=wo_lo[:], in_=woP[64:128, :])

        def emit_v(j):
            """Project V^ for s-chunk j (tokens j*128..j*128+128), no bias."""
            c, sub = divmod(j, 4)
            if sub == 0 and c + 1 < 8:
                load_vraw(c + 1)
            ps = projp.tile([128, 512], f32, tag="proj", name="proj_ps")
            for kk in range(4):
                MM(
                    ps[:, 0:128],
                    vraw_tiles[c][:, kk, sub * 128 : (sub + 1) * 128],
                    wv_t[:, kk, :],
                    start=(kk == 0),
                    stop=(kk == 3),
                    skip_group_check=True,
                )
            vs = vstore.tile([128, HP, DK + 1], bf, tag="vs", name="vs")
            v_store.append(vs)
            nc.vector.memset(vs[:, :, DK : DK + 1], 1.0)
            nc.vector.tensor_copy(
                out=vs[:, :, 0:DK],
                in_=ps[:, 0:128].rearrange("p (h c) -> p h c", c=DK),
            )

        def emit_oproj(T, i):
            """Output-projection partial for tq-tile i of block T -> DRAM."""
            ps = projp.tile([128, 512], f32, tag="proj", name="oproj_ps")
            MM(
                ps[:],
                o_pair[:, T * TQB + i * 128 : T * TQB + (i + 1) * 128],
                wo_t[:],
                start=True,
                stop=True,
                skip_group_check=True,
            )
            ot = ostage.tile([128, 512], bf, tag="ot", name="ot")
            nc.vector.tensor_copy(out=ot[:], in_=ps[:])
            dma(out=out_p[T * TQB + i * 128 : T * TQB + (i + 1) * 128, :], in_=ot[:])

        def make_norm_steps(T, h, pvsb):
            """Closures normalizing (T, h)'s output from its SBUF copies."""
            steps = []
            for half in range(2):
                i = 4 * T + 2 * h + half

                def s1(i=i, pv1=pvsb[half]):
                    # bounce through DRAM to spread the denominator row over
                    # 64 partitions: DVE reciprocal is ~6.5 cycles per FREE
                    # element, so shape [64, 8] beats [1, 512] by ~15x
                    dma(out=rden[i : i + 1, :], in_=pv1[64:65, :])
                    sp = wspool.tile([64, 8], bf, tag="sp", name="sp")
                    dma(out=sp[:], in_=rden[i].rearrange("(p e) -> p e", p=64))
                    sp2 = wspool.tile([64, 8], f32, tag="sp2", name="sp2")
                    nc.vector.reciprocal(out=sp2[:], in_=sp[:])
                    dma(out=rrec[i].rearrange("(p e) -> p e", p=64), in_=sp2[:])

                def s2(i=i, half=half, h=h, T=T, pv1=pvsb[half]):
                    w = wspool.tile([64, 512], f32, tag="ws", name="wst")
                    dma(out=w[:], in_=rrec[i : i + 1, :].partition_broadcast(64))
                    nc.gpsimd.tensor_mul(
                        out=o_pair[
                            h * 64 : (h + 1) * 64,
                            T * TQB + half * 512 : T * TQB + half * 512 + 512,
                        ],
                        in0=pv1[0:64, :],
                        in1=w[:],
                    )

                steps.append(s1)
                steps.append(s2)
            return steps

        # ---- attention: flat unit stream, scores one chunk ahead ----
        def emit_scores(h, T, j):
            pb = h * 64
            sc = scorep.tile([128, 1024], f32, tag="sc", name="sc")
            for half in range(2):
                MM(
                    sc[:, half * 512 : half * 512 + 512],
                    kt[pb : pb + 64, j * 128 : (j + 1) * 128],
                    qt[pb : pb + 64, T * TQB + half * 512 : T * TQB + half * 512 + 512],
                    start=True,
                    stop=True,
                    skip_group_check=True,
                )
            return sc

        # prologue: kick all first-need DMAs in parallel, warm the PE on
        # dummy data while they land, then project K chunk 0 + Q chunks 0-1
        rk0 = load_raw(kT, 0, eng=nc.scalar)
        rq0 = load_raw(qT, 0)
        rq1 = load_raw(qT, 1, eng=nc.scalar)
        load_wv()
        load_vraw(0)
        for _ in range(12):
            wps = projp.tile([128, 512], f32, tag="proj", name="warm_ps")
            MM(
                wps[:],
                dummy[0:64, 0:128],
                dummy[0:64, :],
                start=True,
                stop=True,
                skip_group_check=True,
            )
        proj_from_raw(rk0, wk_t, bk_t, kt, 0)
        proj_from_raw(rq0, wq_t, bq_t, qt, 0)
        proj_from_raw(rq1, wq_t, bq_t, qt, 1)
        emit_v(0)

        # per-unit extra-work schedule: (h, T, j) -> list of closures
        extra = {}
        # segment (0,0): K chunks 1-7 (chunk c needed by scores j=4c, emitted
        # ~2 ahead), Q chunks 2-7 (needed from T=1 on)
        for n, jj in enumerate((2, 6, 10, 14, 18, 22, 26)):
            extra.setdefault((0, 0, jj), []).append(
                lambda c=n + 1: proj_chunk(kT, wk_t, bk_t, kt, c)
            )

        for n, jj in enumerate((3, 7, 11, 15, 19, 23)):
            extra.setdefault((1, 0, jj), []).append(
                lambda c=n + 2: proj_chunk(qT, wq_t, bq_t, qt, c)
            )
        extra[(1, 0, 1)] = [load_wo]

        # Last block (T=3): head-0 half of the out-projection runs inside the
        # (h=1) unit stream (slice i's columns are normalized by then); the
        # result is staged in SBUF so the tail only needs the head-1 half.
        pa_sb_tiles = []

        def emit_pa(i):
            col = (NT - 1) * TQB + i * 128
            ps = projp.tile([128, 512], f32, tag="proj", name="pa_ps")
            MM(
                ps[:],
                o_pair[0:64, col : col + 128],
                wo_t[0:64, :],
                start=True,
                stop=True,
                skip_group_check=True,
            )
            t = pasb.tile([128, 512], f32, tag="pasb", name="pa_sb")
            pa_sb_tiles.append(t)
            nc.vector.tensor_copy(out=t[:], in_=ps[:])

        for n, jj in enumerate((12, 14, 18, 20, 24, 26, 28, 30)):
            extra.setdefault((1, NT - 1, jj), []).append(lambda i=n: emit_pa(i))

        pend_norm = {}  # (T, h) -> norm step closures, filled as units finish
        norm_slots = {4: 0, 10: 1, 16: 2, 22: 3}
        oproj_slots = (2, 6, 9, 13, 17, 21, 25, 29)

        units = [(h, T, j) for T in range(NT) for h in range(HP) for j in range(32)]
        pv_tiles = {}
        pvsb_tiles = {}

        sc_next = emit_scores(0, 0, 0)
        for idx, (h, T, j) in enumerate(units):
            if j == 0:
                pv_tiles[(T, h)] = [
                    pvp.tile([DK + 1, 512], f32, tag="pv", name=f"pv{_h}")
                    for _h in range(2)
                ]
            sc = sc_next
            pt = ppool.tile([128, 1024], bf, tag="pt", name="pt")
            nc.scalar.activation(out=pt[:], in_=sc[:], func=Exp, scale=0.125)
            # next unit's scores go to PE first so ACT(u+1) is never gated
            # by interleaved projection work
            if idx + 1 < len(units):
                nh, nT, nj = units[idx + 1]
                sc_next = emit_scores(nh, nT, nj)
            # interleaved extra work (runs on PE/DVE/DMA while ACT is busy)
            if h == 0 and T == 0 and j + 1 < 32:
                emit_v(j + 1)
            for fn in extra.get((h, T, j), ()):
                fn()
            # normalization of the previous (T, h) segment
            prev = (T, h - 1) if h > 0 else (T - 1, HP - 1)
            if j in norm_slots and prev in pend_norm:
                pend_norm[prev][norm_slots[j]]()
            # output projection of block T-1 during (T, h=1)
            if h == 1 and T >= 1 and j in oproj_slots:
                emit_oproj(T - 1, oproj_slots.index(j))
            pv = pv_tiles[(T, h)]
            for half in range(2):
                MM(
                    pv[half][:],
                    v_store[j][:, h, :],
                    pt[:, half * 512 : half * 512 + 512],
                    start=(j == 0),
                    stop=(j == 31),
                    skip_group_check=True,
                )
            if j == 31:
                pvsb = []
                for half in range(2):
                    t = wspool.tile([DK + 1, 512], bf, tag="pvsb", name="pvsb")
                    nc.vector.tensor_copy(out=t[:], in_=pv[half][:])
                    pvsb.append(t)
                pvsb_tiles[(T, h)] = pvsb
                if not (T == NT - 1 and h == HP - 1):
                    pend_norm[(T, h)] = make_norm_steps(T, h, pvsb)

        # Tail: block T=3. Head 0's half already ran inside (3,1)'s stream
        # (pa_sb tiles). Head 1 skips normalization: out uses the
        # UNNORMALIZED O1 (bf16 copy of pvsb) and the reciprocal is applied
        # per-partition after the out-projection.
        Tl = NT - 1
        pvsb1 = pvsb_tiles[(Tl, 1)]
        ps_rc = projp.tile([128, 512], f32, tag="proj", name="ps_rc")
        for i in range(8):
            MM(
                ps_rc[:, i : i + 1],
                pvsb1[i // 4][64:65, (i % 4) * 128 : (i % 4) * 128 + 128],
                ones_bf[64:65, :],
                start=True,
                stop=True,
                skip_group_check=True,
            )
        rc_all = wspool.tile([128, 8], f32, tag="rcall2", name="rc_all")
        nc.vector.reciprocal(out=rc_all[:], in_=ps_rc[:, 0:8])

        for i in range(8):
            col = Tl * TQB + i * 128
            pb = projp.tile([128, 512], f32, tag="proj", name="out_pb")
            MM(
                pb[:],
                pvsb1[i // 4][0:64, (i % 4) * 128 : (i % 4) * 128 + 128],
                wo_lo[:],
                start=True,
                stop=True,
                skip_group_check=True,
            )
            ot = ostage.tile([128, 512], bf, tag="ot", name="ot")
            nc.vector.scalar_tensor_tensor(
                out=ot[:],
                in0=pb[:],
                scalar=rc_all[:, i : i + 1],
                in1=pa_sb_tiles[i][:],
                op0=Mult,
                op1=Add,
            )
            dma(out=out_p[col : col + 128, :], in_=ot[:])

    if not nc.is_finalized():
        nc.finalize()
    return nc


def _get_program():
    global _PROGRAM
    if _PROGRAM is None:
        _PROGRAM = _build_program()
    return _PROGRAM


def _prep_inputs(q, k, v, w_q, b_q, w_k, b_k, w_v, b_v, w_o, b_o):
    bf16 = ml_dtypes.bfloat16
    q = np.asarray(q, dtype=np.float32)
    k = np.asarray(k, dtype=np.float32)
    v = np.asarray(v, dtype=np.float32)
    qT = [np.ascontiguousarray(q[b].T).astype(bf16) for b in range(B)]  # [D, S]
    kT = [np.ascontiguousarray(k[b].T).astype(bf16) for b in range(B)]
    vT = [np.ascontiguousarray(v[b].T).astype(bf16) for b in range(B)]
    w_q = np.asarray(w_q, np.float32)
    w_k = np.asarray(w_k, np.float32)
    w_v = np.asarray(w_v, np.float32)
    w_o = np.asarray(w_o, np.float32)

    in_maps = []
    for c in range(N_CORES):
        b, p = divmod(c, 4)
        P = slice(128 * p, 128 * p + 128)
        in_maps.append(
            {
                "qT": qT[b],
                "kT": kT[b],
                "vT": vT[b],
                "wqPT": np.ascontiguousarray(w_q[P, :].T).astype(bf16),
                "wkPT": np.ascontiguousarray(w_k[P, :].T).astype(bf16),
                "wvPT": np.ascontiguousarray(w_v[P, :].T).astype(bf16),
                "woP": np.ascontiguousarray(w_o[:, P].T).astype(bf16),
                "bqPc": np.ascontiguousarray(
                    np.asarray(b_q, np.float32)[P].reshape(128, 1)
                ),
                "bkPc": np.ascontiguousarray(
                    np.asarray(b_k, np.float32)[P].reshape(128, 1)
                ),
            }
        )
    return in_maps


def run_cores(in_maps, trace=False, **kw):
    """Compile+run the SPMD program; returns BassKernelResults."""
    from concourse.bass_utils import run_bass_kernel_spmd

    nc = _get_program()
    return run_bass_kernel_spmd(nc, in_maps, list(range(N_CORES)), trace=trace, **kw)


def _combine(results, b_v, b_o, w_o):
    bo_eff = (
        np.asarray(b_o, np.float32)
        + np.asarray(b_v, np.float32) @ np.asarray(w_o, np.float32).T
    )
    out = np.empty((B, S, D), np.float32)
    for b in range(B):
        acc = results[4 * b]["out"].astype(np.float32)
        for p in range(1, 4):
            acc += results[4 * b + p]["out"].astype(np.float32)
        out[b] = acc + bo_eff
    return out


def kernel(q, k, v, w_q, b_q, w_k, b_k, w_v, b_v, w_o, b_o):
    in_maps = _prep_inputs(q, k, v, w_q, b_q, w_k, b_k, w_v, b_v, w_o, b_o)
    res = run_cores(in_maps)
    return _combine(res.results, b_v, b_o, w_o)


# revision 46
# speedup vs baseline: 1.1730x; 1.1730x over previous
"""Multi-head attention (B=2, S=4096, D=512, H=8) on 8 TRN2 NeuronCores.

Sharding: batch x head-pair. Core c handles batch b=c//4, head pair
p=c%4 (heads 2p, 2p+1 = model dims 128p..128p+128), and ALL 4096 query
tokens of its batch. Each core projects only its own 128-dim slice of
Q^/K^/V^ (no duplicated projection work across the batch group), runs
flash-style attention for its 2 heads over the full 4096x4096 score
block, and emits the PARTIAL output projection O_pair^T @ W_o[pair, :]
in bf16 for all 4096 tokens. The host sums the 4 partials per batch
and adds the effective bias (b_o + b_v @ W_o^T), so no cross-core
communication is needed on device.

All matmuls in bf16 with fp32 PSUM accumulation. Softmax skips the
max-subtraction (scores ~ N(0,1); exp is safe in fp32) and the
denominator comes from a ones column appended to V^, so softmax costs
exactly one ACT pass per score tile. Q/K biases are folded into the
PSUM->SBUF copy on the vector engine (zero bias matmuls).

Pipelining: scores for unit u+1 are emitted before PV of unit u and
ahead of any interleaved projection work, so ACT (the pacing engine in
steady state, ~1.1us per 128x1024 exp) never waits; K/Q/V projections,
normalization chains, and output-projection tiles ride the unit
stream. The last block's second head skips normalization: its
out-projection runs unnormalized and the reciprocal - produced by
transposing the denominator row into PSUM columns with K=1 matmuls
(no DMA round trip) - is applied per-partition in the final
scalar_tensor_tensor. The tensor engine is pre-warmed on dummy
matmuls during the prologue DMA window (p-state ramp).
"""

import numpy as np
import ml_dtypes

B, S, D = 2, 4096, 512
H, DK = 8, 64
N_CORES = 8
HP = 2  # heads per core (one pair)
NT = 4  # tq blocks of 1024
TQB = 1024

_PROGRAM = None


def _build_program():
    from contextlib import ExitStack

    import concourse.mybir as mybir
    import concourse.tile as tile
    from concourse import bacc

    bf = mybir.dt.bfloat16
    f32 = mybir.dt.float32
    Exp = mybir.ActivationFunctionType.Exp
    Add = mybir.AluOpType.add
    Mult = mybir.AluOpType.mult
    Bypass = mybir.AluOpType.bypass

    nc = bacc.Bacc(None)

    qT = nc.declare_dram_parameter("qT", [D, S], bf, isOutput=False)
    kT = nc.declare_dram_parameter("kT", [D, S], bf, isOutput=False)
    vT = nc.declare_dram_parameter("vT", [D, S], bf, isOutput=False)
    wqPT = nc.declare_dram_parameter("wqPT", [D, 128], bf, isOutput=False)
    wkPT = nc.declare_dram_parameter("wkPT", [D, 128], bf, isOutput=False)
    wvPT = nc.declare_dram_parameter("wvPT", [D, 128], bf, isOutput=False)
    woP = nc.declare_dram_parameter("woP", [128, D], bf, isOutput=False)
    bqPc = nc.declare_dram_parameter("bqPc", [128, 1], f32, isOutput=False)
    bkPc = nc.declare_dram_parameter("bkPc", [128, 1], f32, isOutput=False)
    out_p = nc.declare_dram_parameter("out", [S, D], bf, isOutput=True)
    # DRAM scratch rows for softmax denominator / reciprocal broadcasting
    rden = nc.dram_tensor("rden", [HP * NT * 2, 512], bf)
    rrec = nc.dram_tensor("rrec", [HP * NT * 2, 512], f32)

    with tile.TileContext(nc) as tc, ExitStack() as ctx:
        wpool = ctx.enter_context(tc.tile_pool(name="w", bufs=1))
        qstream = ctx.enter_context(tc.tile_pool(name="qstream", bufs=2))
        kstream = ctx.enter_context(tc.tile_pool(name="kstream", bufs=2))
        vstream = ctx.enter_context(tc.tile_pool(name="vstream", bufs=2))
        qtres = ctx.enter_context(tc.tile_pool(name="qtres", bufs=1))
        ktres = ctx.enter_context(tc.tile_pool(name="ktres", bufs=1))
        vstore = ctx.enter_context(tc.tile_pool(name="vstore", bufs=32))
        ppool = ctx.enter_context(tc.tile_pool(name="p", bufs=4))
        opool = ctx.enter_context(tc.tile_pool(name="o", bufs=1))
        wspool = ctx.enter_context(tc.tile_pool(name="ws", bufs=4))
        ostage = ctx.enter_context(tc.tile_pool(name="ostage", bufs=3))
        pasb = ctx.enter_context(tc.tile_pool(name="pasb", bufs=8))
        projp = ctx.enter_context(tc.tile_pool(name="projp", bufs=2, space="PSUM"))
        scorep = ctx.enter_context(tc.tile_pool(name="scorep", bufs=2, space="PSUM"))
        pvp = ctx.enter_context(tc.tile_pool(name="pvp", bufs=2, space="PSUM"))

        dma = nc.sync.dma_start
        MM = nc.tensor.matmul

        def wtile(param, tagp):
            t = wpool.tile([128, 4, 128], bf, tag=tagp, name=tagp)
            dma(out=t[:], in_=param[:].rearrange("(c p) m -> p c m", p=128))
            return t

        # ---- essential constants first (everything else is deferred) ----
        wq_t = wtile(wqPT, "wq")
        bq_t = wpool.tile([128, 1], f32, tag="bq", name="bq_t")
        dma(out=bq_t[:], in_=bqPc[:])
        wk_t = wtile(wkPT, "wk")
        bk_t = wpool.tile([128, 1], f32, tag="bk", name="bk_t")
        dma(out=bk_t[:], in_=bkPc[:])
        # dummy SBUF operand for scalar_tensor_tensor bias adds (op1=bypass
        # ignores it, but only one non-scalar input may come from PSUM)
        dummy = wpool.tile([128, 512], bf, tag="dummy", name="dummy")
        nc.vector.memset(dummy[:], 0.0)
        ones_bf = wpool.tile([128, 1], bf, tag="ones1", name="ones_bf")
        nc.vector.memset(ones_bf[:], 1.0)

        qt = qtres.tile([128, S], bf, tag="qt", name="qt")
        kt = ktres.tile([128, S], bf, tag="kt", name="kt")
        o_pair = opool.tile([128, S], bf, tag="o", name="o_pair")
        v_store = []  # 32 tiles [128, HP, DK+1]; last col per head = ones

        def load_raw(src, c, eng=None):
            raw = (qstream if src is qT else kstream).tile(
                [128, 4, 512], bf, tag="raw", name="raw"
            )
            (eng.dma_start if eng is not None else dma)(
                out=raw[:],
                in_=src[:, c * 512 : (c + 1) * 512].rearrange(
                    "(c p) t -> p c t", p=128
                ),
            )
            return raw

        def proj_from_raw(raw, w_t, b_t, dst, c):
            ps = projp.tile([128, 512], f32, tag="proj", name="proj_ps")
            for kk in range(4):
                MM(
                    ps[:],
                    w_t[:, kk, :],
                    raw[:, kk, :],
                    start=(kk == 0),
                    stop=(kk == 3),
                    skip_group_check=True,
                )
            nc.vector.scalar_tensor_tensor(
                out=dst[:, c * 512 : (c + 1) * 512],
                in0=ps[:],
                scalar=b_t[:],
                in1=dummy[:],
                op0=Add,
                op1=Bypass,
            )

        def proj_chunk(src, w_t, b_t, dst, c):
            proj_from_raw(load_raw(src, c), w_t, b_t, dst, c)

        # ---- deferred constant loads / computations (closures) ----
        wv_t = None
        vraw_tiles = {}

        def load_wv():
            nonlocal wv_t
            wv_t = wtile(wvPT, "wv")

        def load_vraw(c, eng=None):
            t = vstream.tile([128, 4, 512], bf, tag="vraw", name="vraw")
            (eng.dma_start if eng is not None else dma)(
                out=t[:],
                in_=vT[:, c * 512 : (c + 1) * 512].rearrange(
                    "(c p) t -> p c t", p=128
                ),
            )
            vraw_tiles[c] = t

        wo_t = wo_lo = None

        def load_wo():
            nonlocal wo_t, wo_lo
            wo_t = wpool.tile([128, D], bf, tag="wo", name="wo_t")
            dma(out=wo_t[:], in_=woP[:])
            # head-1 rows staged at base partition 0 so the tail pb matmuls
            # can pair them with pvsb tiles (also at partition 0)
            wo_lo = wpool.tile([64, D], bf, tag="wolo", name="wo_lo")
            dma(out=wo_lo[:], in_=woP[64:128, :])

        def emit_v(j):
            """Project V^ for s-chunk j (tokens j*128..j*128+128), no bias."""
            c, sub = divmod(j, 4)
            if sub == 0 and c + 1 < 8:
                load_vraw(c + 1)
            ps = projp.tile([128, 512], f32, tag="proj", name="proj_ps")
            for kk in range(4):
                MM(
                    ps[:, 0:128],
                    vraw_tiles[c][:, kk, sub * 128 : (sub + 1) * 128],
                    wv_t[:, kk, :],
                    start=(kk == 0),
                    stop=(kk == 3),
                    skip_group_check=True,
                )
            vs = vstore.tile([128, HP, DK + 1], bf, tag="vs", name="vs")
            v_store.append(vs)
            nc.vector.memset(vs[:, :, DK : DK + 1], 1.0)
            nc.vector.tensor_copy(
                out=vs[:, :, 0:DK],
                in_=ps[:, 0:128].rearrange("p (h c) -> p h c", c=DK),
            )

        def emit_oproj(T, i):
            """Output-projection partial for tq-tile i of block T -> DRAM."""
            ps = projp.tile([128, 512], f32, tag="proj", name="oproj_ps")
            MM(
                ps[:],
                o_pair[:, T * TQB + i * 128 : T * TQB + (i + 1) * 128],
                wo_t[:],
                start=True,
                stop=True,
                skip_group_check=True,
            )
            ot = ostage.tile([128, 512], bf, tag="ot", name="ot")
            nc.vector.tensor_copy(out=ot[:], in_=ps[:])
            dma(out=out_p[T * TQB + i * 128 : T * TQB + (i + 1) * 128, :], in_=ot[:])

        def make_norm_steps(T, h, pvsb):
            """Closures normalizing (T, h)'s output from its SBUF copies."""
            steps = []
            for half in range(2):
                i = 4 * T + 2 * h + half

                def s1(i=i, pv1=pvsb[half]):
                    # bounce through DRAM to spread the denominator row over
                    # 64 partitions: DVE reciprocal is ~6.5 cycles per FREE
                    # element, so shape [64, 8] beats [1, 512] by ~15x
                    dma(out=rden[i : i + 1, :], in_=pv1[64:65, :])
                    sp = wspool.tile([64, 8], bf, tag="sp", name="sp")
                    dma(out=sp[:], in_=rden[i].rearrange("(p e) -> p e", p=64))
                    sp2 = wspool.tile([64, 8], f32, tag="sp2", name="sp2")
                    nc.vector.reciprocal(out=sp2[:], in_=sp[:])
                    dma(out=rrec[i].rearrange("(p e) -> p e", p=64), in_=sp2[:])

                def s2(i=i, half=half, h=h, T=T, pv1=pvsb[half]):
                    w = wspool.tile([64, 512], f32, tag="ws", name="wst")
                    dma(out=w[:], in_=rrec[i : i + 1, :].partition_broadcast(64))
                    nc.gpsimd.tensor_mul(
                        out=o_pair[
                            h * 64 : (h + 1) * 64,
                            T * TQB + half * 512 : T * TQB + half * 512 + 512,
                        ],
                        in0=pv1[0:64, :],
                        in1=w[:],
                    )

                steps.append(s1)
                steps.append(s2)
            return steps

        # ---- attention: flat unit stream, scores one chunk ahead ----
        def emit_scores(h, T, j):
            pb = h * 64
            sc = scorep.tile([128, 1024], f32, tag="sc", name="sc")
            for half in range(2):
                MM(
                    sc[:, half * 512 : half * 512 + 512],
                    kt[pb : pb + 64, j * 128 : (j + 1) * 128],
                    qt[pb : pb + 64, T * TQB + half * 512 : T * TQB + half * 512 + 512],
                    start=True,
                    stop=True,
                    skip_group_check=True,
                )
            return sc

        # prologue: kick all first-need DMAs in parallel, warm the PE on
        # dummy data while they land, then project K chunk 0 + Q chunks 0-1
        rk0 = load_raw(kT, 0, eng=nc.scalar)
        rq0 = load_raw(qT, 0)
        rq1 = load_raw(qT, 1, eng=nc.scalar)
        load_wv()
        load_vraw(0)
        for _ in range(12):
            wps = projp.tile([128, 512], f32, tag="proj", name="warm_ps")
            MM(
                wps[:],
                dummy[0:64, 0:128],
                dummy[0:64, :],
                start=True,
                stop=True,
                skip_group_check=True,
            )
        proj_from_raw(rk0, wk_t, bk_t, kt, 0)
        proj_from_raw(rq0, wq_t, bq_t, qt, 0)
        # first unit's scores half-a as soon as K0+Q0 exist; half-b after Q1.
        # Its exp is split into halves in the loop so half-a isn't gated on Q1.
        sc0 = scorep.tile([128, 1024], f32, tag="sc", name="sc")
        MM(
            sc0[:, 0:512],
            kt[0:64, 0:128],
            qt[0:64, 0:512],
            start=True,
            stop=True,
            skip_group_check=True,
        )
        proj_from_raw(rq1, wq_t, bq_t, qt, 1)
        MM(
            sc0[:, 512:1024],
            kt[0:64, 0:128],
            qt[0:64, 512:1024],
            start=True,
            stop=True,
            skip_group_check=True,
        )
        emit_v(0)

        # per-unit extra-work schedule: (h, T, j) -> list of closures
        extra = {}
        # segment (0,0): K chunks 1-7 (chunk c needed by scores j=4c, emitted
        # ~2 ahead), Q chunks 2-7 (needed from T=1 on)
        for n, jj in enumerate((2, 6, 10, 14, 18, 22, 26)):
            extra.setdefault((0, 0, jj), []).append(
                lambda c=n + 1: proj_chunk(kT, wk_t, bk_t, kt, c)
            )

        for n, jj in enumerate((3, 7, 11, 15, 19, 23)):
            extra.setdefault((1, 0, jj), []).append(
                lambda c=n + 2: proj_chunk(qT, wq_t, bq_t, qt, c)
            )
        extra[(1, 0, 1)] = [load_wo]

        # Last block (T=3): head-0 half of the out-projection runs inside the
        # (h=1) unit stream (slice i's columns are normalized by then); the
        # result is staged in SBUF so the tail only needs the head-1 half.
        pa_sb_tiles = []

        def emit_pa(i):
            col = (NT - 1) * TQB + i * 128
            ps = projp.tile([128, 512], f32, tag="proj", name="pa_ps")
            MM(
                ps[:],
                o_pair[0:64, col : col + 128],
                wo_t[0:64, :],
                start=True,
                stop=True,
                skip_group_check=True,
            )
            t = pasb.tile([128, 512], f32, tag="pasb", name="pa_sb")
            pa_sb_tiles.append(t)
            nc.vector.tensor_copy(out=t[:], in_=ps[:])

        for n, jj in enumerate((12, 14, 18, 20, 24, 26, 28, 30)):
            extra.setdefault((1, NT - 1, jj), []).append(lambda i=n: emit_pa(i))

        pend_norm = {}  # (T, h) -> norm step closures, filled as units finish
        norm_slots = {4: 0, 10: 1, 16: 2, 22: 3}
        oproj_slots = (2, 6, 9, 13, 17, 21, 25, 29)

        units = [(h, T, j) for T in range(NT) for h in range(HP) for j in range(32)]
        pv_tiles = {}
        pvsb_tiles = {}

        sc_next = sc0
        for idx, (h, T, j) in enumerate(units):
            if j == 0:
                pv_tiles[(T, h)] = [
                    pvp.tile([DK + 1, 512], f32, tag="pv", name=f"pv{_h}")
                    for _h in range(2)
                ]
            sc = sc_next
            pt = ppool.tile([128, 1024], bf, tag="pt", name="pt")
            if idx == 0:
                for hf in range(2):
                    nc.scalar.activation(
                        out=pt[:, hf * 512 : hf * 512 + 512],
                        in_=sc[:, hf * 512 : hf * 512 + 512],
                        func=Exp,
                        scale=0.125,
                    )
            else:
                nc.scalar.activation(out=pt[:], in_=sc[:], func=Exp, scale=0.125)
            # next unit's scores go to PE first so ACT(u+1) is never gated
            # by interleaved projection work
            if idx + 1 < len(units):
                nh, nT, nj = units[idx + 1]
                sc_next = emit_scores(nh, nT, nj)
            # interleaved extra work (runs on PE/DVE/DMA while ACT is busy)
            if h == 0 and T == 0 and j + 1 < 32:
                emit_v(j + 1)
            for fn in extra.get((h, T, j), ()):
                fn()
            # normalization of the previous (T, h) segment
            prev = (T, h - 1) if h > 0 else (T - 1, HP - 1)
            if j in norm_slots and prev in pend_norm:
                pend_norm[prev][norm_slots[j]]()
            # output projection of block T-1 during (T, h=1)
            if h == 1 and T >= 1 and j in oproj_slots:
                emit_oproj(T - 1, oproj_slots.index(j))
            pv = pv_tiles[(T, h)]
            for half in range(2):
                MM(
                    pv[half][:],
                    v_store[j][:, h, :],
                    pt[:, half * 512 : half * 512 + 512],
                    start=(j == 0),
                    stop=(j == 31),
                    skip_group_check=True,
                )
            if j == 31:
                pvsb = []
                for half in range(2):
                    t = wspool.tile([DK + 1, 512], bf, tag="pvsb", name="pvsb")
                    nc.vector.tensor_copy(out=t[:], in_=pv[half][:])
                    pvsb.append(t)
                pvsb_tiles[(T, h)] = pvsb
                if not (T == NT - 1 and h == HP - 1):
                    pend_norm[(T, h)] = make_norm_steps(T, h, pvsb)

        # Tail: block T=3. Head 0's half already ran inside (3,1)'s stream
        # (pa_sb tiles). Head 1 skips normalization: out uses the
        # UNNORMALIZED O1 (bf16 copy of pvsb) and the reciprocal is applied
        # per-partition after the out-projection.
        Tl = NT - 1
        pvsb1 = pvsb_tiles[(Tl, 1)]
        ps_rc = projp.tile([128, 512], f32, tag="proj", name="ps_rc")
        for i in range(8):
            MM(
                ps_rc[:, i : i + 1],
                pvsb1[i // 4][64:65, (i % 4) * 128 : (i % 4) * 128 + 128],
                ones_bf[64:65, :],
                start=True,
                stop=True,
                skip_group_check=True,
            )
        rc_all = wspool.tile([128, 8], f32, tag="rcall2", name="rc_all")
        nc.vector.reciprocal(out=rc_all[:], in_=ps_rc[:, 0:8])

        for i in range(8):
            col = Tl * TQB + i * 128
            pb = projp.tile([128, 512], f32, tag="proj", name="out_pb")
            MM(
                pb[:],
                pvsb1[i // 4][0:64, (i % 4) * 128 : (i % 4) * 128 + 128],
                wo_lo[:],
                start=True,
                stop=True,
                skip_group_check=True,
            )
            ot = ostage.tile([128, 512], bf, tag="ot", name="ot")
            nc.vector.scalar_tensor_tensor(
                out=ot[:],
                in0=pb[:],
                scalar=rc_all[:, i : i + 1],
                in1=pa_sb_tiles[i][:],
                op0=Mult,
                op1=Add,
            )
            dma(out=out_p[col : col + 128, :], in_=ot[:])

    if not nc.is_finalized():
        nc.finalize()
    return nc


def _get_program():
    global _PROGRAM
    if _PROGRAM is None:
        _PROGRAM = _build_program()
    return _PROGRAM


def _prep_inputs(q, k, v, w_q, b_q, w_k, b_k, w_v, b_v, w_o, b_o):
    bf16 = ml_dtypes.bfloat16
    q = np.asarray(q, dtype=np.float32)
    k = np.asarray(k, dtype=np.float32)
    v = np.asarray(v, dtype=np.float32)
    qT = [np.ascontiguousarray(q[b].T).astype(bf16) for b in range(B)]  # [D, S]
    kT = [np.ascontiguousarray(k[b].T).astype(bf16) for b in range(B)]
    vT = [np.ascontiguousarray(v[b].T).astype(bf16) for b in range(B)]
    w_q = np.asarray(w_q, np.float32)
    w_k = np.asarray(w_k, np.float32)
    w_v = np.asarray(w_v, np.float32)
    w_o = np.asarray(w_o, np.float32)

    in_maps = []
    for c in range(N_CORES):
        b, p = divmod(c, 4)
        P = slice(128 * p, 128 * p + 128)
        in_maps.append(
            {
                "qT": qT[b],
                "kT": kT[b],
                "vT": vT[b],
                "wqPT": np.ascontiguousarray(w_q[P, :].T).astype(bf16),
                "wkPT": np.ascontiguousarray(w_k[P, :].T).astype(bf16),
                "wvPT": np.ascontiguousarray(w_v[P, :].T).astype(bf16),
                "woP": np.ascontiguousarray(w_o[:, P].T).astype(bf16),
                "bqPc": np.ascontiguousarray(
                    np.asarray(b_q, np.float32)[P].reshape(128, 1)
                ),
                "bkPc": np.ascontiguousarray(
                    np.asarray(b_k, np.float32)[P].reshape(128, 1)
                ),
            }
        )
    return in_maps


def run_cores(in_maps, trace=False, **kw):
    """Compile+run the SPMD program; returns BassKernelResults."""
    from concourse.bass_utils import run_bass_kernel_spmd

    nc = _get_program()
    return run_bass_kernel_spmd(nc, in_maps, list(range(N_CORES)), trace=trace, **kw)


def _combine(results, b_v, b_o, w_o):
    bo_eff = (
        np.asarray(b_o, np.float32)
        + np.asarray(b_v, np.float32) @ np.asarray(w_o, np.float32).T
    )
    out = np.empty((B, S, D), np.float32)
    for b in range(B):
        acc = results[4 * b]["out"].astype(np.float32)
        for p in range(1, 4):
            acc += results[4 * b + p]["out"].astype(np.float32)
        out[b] = acc + bo_eff
    return out


def kernel(q, k, v, w_q, b_q, w_k, b_k, w_v, b_v, w_o, b_o):
    in_maps = _prep_inputs(q, k, v, w_q, b_q, w_k, b_k, w_v, b_v, w_o, b_o)
    res = run_cores(in_maps)
    return _combine(res.results, b_v, b_o, w_o)


# revision 47
# speedup vs baseline: 1.1847x; 1.0100x over previous
"""Multi-head attention (B=2, S=4096, D=512, H=8) on 8 TRN2 NeuronCores.

Sharding: batch x head-pair. Core c handles batch b=c//4, head pair
p=c%4 (heads 2p, 2p+1 = model dims 128p..128p+128), and ALL 4096 query
tokens of its batch. Each core projects only its own 128-dim slice of
Q^/K^/V^ (no duplicated projection work across the batch group), runs
flash-style attention for its 2 heads over the full 4096x4096 score
block, and emits the PARTIAL output projection O_pair^T @ W_o[pair, :]
in bf16 for all 4096 tokens. The host sums the 4 partials per batch
and adds the effective bias (b_o + b_v @ W_o^T), so no cross-core
communication is needed on device.

All matmuls in bf16 with fp32 PSUM accumulation. Softmax skips the
max-subtraction (scores ~ N(0,1); exp is safe in fp32) and the
denominator comes from a ones column appended to V^, so softmax costs
exactly one ACT pass per score tile. Q/K biases are folded into the
PSUM->SBUF copy on the vector engine (zero bias matmuls).

Pipelining: scores for unit u+1 are emitted before PV of unit u and
ahead of any interleaved projection work, so ACT (the pacing engine in
steady state, ~1.1us per 128x1024 exp) never waits; K/Q/V projections,
normalization chains, and output-projection tiles ride the unit
stream. The last block's second head skips normalization: its
out-projection runs unnormalized and the reciprocal - produced by
transposing the denominator row into PSUM columns with K=1 matmuls
(no DMA round trip) - is applied per-partition in the final
scalar_tensor_tensor. The tensor engine is pre-warmed on dummy
matmuls during the prologue DMA window (p-state ramp).
"""

import numpy as np
import ml_dtypes

B, S, D = 2, 4096, 512
H, DK = 8, 64
N_CORES = 8
HP = 2  # heads per core (one pair)
NT = 4  # tq blocks of 1024
TQB = 1024

_PROGRAM = None


def _build_program():
    from contextlib import ExitStack

    import concourse.mybir as mybir
    import concourse.tile as tile
    from concourse import bacc

    bf = mybir.dt.bfloat16
    f32 = mybir.dt.float32
    Exp = mybir.ActivationFunctionType.Exp
    Add = mybir.AluOpType.add
    Mult = mybir.AluOpType.mult
    Bypass = mybir.AluOpType.bypass

    nc = bacc.Bacc(None)

    qT = nc.declare_dram_parameter("qT", [D, S], bf, isOutput=False)
    kT = nc.declare_dram_parameter("kT", [D, S], bf, isOutput=False)
    vT = nc.declare_dram_parameter("vT", [D, S], bf, isOutput=False)
    wqPT = nc.declare_dram_parameter("wqPT", [D, 128], bf, isOutput=False)
    wkPT = nc.declare_dram_parameter("wkPT", [D, 128], bf, isOutput=False)
    wvPT = nc.declare_dram_parameter("wvPT", [D, 128], bf, isOutput=False)
    woP = nc.declare_dram_parameter("woP", [128, D], bf, isOutput=False)
    bqPc = nc.declare_dram_parameter("bqPc", [128, 1], f32, isOutput=False)
    bkPc = nc.declare_dram_parameter("bkPc", [128, 1], f32, isOutput=False)
    out_p = nc.declare_dram_parameter("out", [S, D], bf, isOutput=True)
    # DRAM scratch rows for softmax denominator / reciprocal broadcasting
    rden = nc.dram_tensor("rden", [HP * NT * 2, 512], bf)
    rrec = nc.dram_tensor("rrec", [HP * NT * 2, 512], f32)

    with tile.TileContext(nc) as tc, ExitStack() as ctx:
        wpool = ctx.enter_context(tc.tile_pool(name="w", bufs=1))
        qstream = ctx.enter_context(tc.tile_pool(name="qstream", bufs=2))
        kstream = ctx.enter_context(tc.tile_pool(name="kstream", bufs=2))
        vstream = ctx.enter_context(tc.tile_pool(name="vstream", bufs=2))
        qtres = ctx.enter_context(tc.tile_pool(name="qtres", bufs=1))
        ktres = ctx.enter_context(tc.tile_pool(name="ktres", bufs=1))
        vstore = ctx.enter_context(tc.tile_pool(name="vstore", bufs=32))
        ppool = ctx.enter_context(tc.tile_pool(name="p", bufs=4))
        opool = ctx.enter_context(tc.tile_pool(name="o", bufs=1))
        wspool = ctx.enter_context(tc.tile_pool(name="ws", bufs=4))
        ostage = ctx.enter_context(tc.tile_pool(name="ostage", bufs=3))
        pasb = ctx.enter_context(tc.tile_pool(name="pasb", bufs=8))
        projp = ctx.enter_context(tc.tile_pool(name="projp", bufs=2, space="PSUM"))
        scorep = ctx.enter_context(tc.tile_pool(name="scorep", bufs=2, space="PSUM"))
        pvp = ctx.enter_context(tc.tile_pool(name="pvp", bufs=2, space="PSUM"))

        dma = nc.sync.dma_start
        MM = nc.tensor.matmul

        def wtile(param, tagp):
            t = wpool.tile([128, 4, 128], bf, tag=tagp, name=tagp)
            dma(out=t[:], in_=param[:].rearrange("(c p) m -> p c m", p=128))
            return t

        # ---- essential constants first (everything else is deferred) ----
        wq_t = wtile(wqPT, "wq")
        bq_t = wpool.tile([128, 1], f32, tag="bq", name="bq_t")
        dma(out=bq_t[:], in_=bqPc[:])
        wk_t = wtile(wkPT, "wk")
        bk_t = wpool.tile([128, 1], f32, tag="bk", name="bk_t")
        dma(out=bk_t[:], in_=bkPc[:])
        # dummy SBUF operand for scalar_tensor_tensor bias adds (op1=bypass
        # ignores it, but only one non-scalar input may come from PSUM)
        dummy = wpool.tile([128, 512], bf, tag="dummy", name="dummy")
        nc.vector.memset(dummy[:], 0.0)
        ones_bf = wpool.tile([128, 1], bf, tag="ones1", name="ones_bf")
        nc.vector.memset(ones_bf[:], 1.0)

        qt = qtres.tile([128, S], bf, tag="qt", name="qt")
        kt = ktres.tile([128, S], bf, tag="kt", name="kt")
        o_pair = opool.tile([128, S], bf, tag="o", name="o_pair")
        v_store = []  # 32 tiles [128, HP, DK+1]; last col per head = ones

        def load_raw(src, c, eng=None):
            raw = (qstream if src is qT else kstream).tile(
                [128, 4, 512], bf, tag="raw", name="raw"
            )
            (eng.dma_start if eng is not None else dma)(
                out=raw[:],
                in_=src[:, c * 512 : (c + 1) * 512].rearrange(
                    "(c p) t -> p c t", p=128
                ),
            )
            return raw

        def proj_from_raw(raw, w_t, b_t, dst, c):
            ps = projp.tile([128, 512], f32, tag="proj", name="proj_ps")
            for kk in range(4):
                MM(
                    ps[:],
                    w_t[:, kk, :],
                    raw[:, kk, :],
                    start=(kk == 0),
                    stop=(kk == 3),
                    skip_group_check=True,
                )
            nc.vector.scalar_tensor_tensor(
                out=dst[:, c * 512 : (c + 1) * 512],
                in0=ps[:],
                scalar=b_t[:],
                in1=dummy[:],
                op0=Add,
                op1=Bypass,
            )

        def proj_chunk(src, w_t, b_t, dst, c):
            proj_from_raw(load_raw(src, c), w_t, b_t, dst, c)

        # ---- deferred constant loads / computations (closures) ----
        wv_t = None
        vraw_tiles = {}

        def load_wv():
            nonlocal wv_t
            wv_t = wtile(wvPT, "wv")

        def load_vraw(c, eng=None):
            t = vstream.tile([128, 4, 512], bf, tag="vraw", name="vraw")
            (eng.dma_start if eng is not None else dma)(
                out=t[:],
                in_=vT[:, c * 512 : (c + 1) * 512].rearrange(
                    "(c p) t -> p c t", p=128
                ),
            )
            vraw_tiles[c] = t

        wo_t = wo_lo = None

        def load_wo():
            nonlocal wo_t, wo_lo
            wo_t = wpool.tile([128, D], bf, tag="wo", name="wo_t")
            dma(out=wo_t[:], in_=woP[:])
            # head-1 rows staged at base partition 0 so the tail pb matmuls
            # can pair them with pvsb tiles (also at partition 0)
            wo_lo = wpool.tile([64, D], bf, tag="wolo", name="wo_lo")
            dma(out=wo_lo[:], in_=woP[64:128, :])

        def emit_v(j):
            """Project V^ for s-chunk j (tokens j*128..j*128+128), no bias."""
            c, sub = divmod(j, 4)
            if sub == 0 and c + 1 < 8:
                load_vraw(c + 1)
            ps = projp.tile([128, 512], f32, tag="proj", name="proj_ps")
            for kk in range(4):
                MM(
                    ps[:, 0:128],
                    vraw_tiles[c][:, kk, sub * 128 : (sub + 1) * 128],
                    wv_t[:, kk, :],
                    start=(kk == 0),
                    stop=(kk == 3),
                    skip_group_check=True,
                )
            vs = vstore.tile([128, HP, DK + 1], bf, tag="vs", name="vs")
            v_store.append(vs)
            nc.gpsimd.memset(vs[:, :, DK : DK + 1], 1.0)
            # alternate the PSUM drain between DVE and ACT (same act-func
            # set as Exp, so no table reload) so consecutive projp ring
            # slots free via different engines and never stall the PE
            if j % 2:
                nc.vector.tensor_copy(
                    out=vs[:, :, 0:DK],
                    in_=ps[:, 0:128].rearrange("p (h c) -> p h c", c=DK),
                )
            else:
                nc.scalar.copy(
                    out=vs[:, :, 0:DK],
                    in_=ps[:, 0:128].rearrange("p (h c) -> p h c", c=DK),
                )

        def emit_oproj(T, i):
            """Output-projection partial for tq-tile i of block T -> DRAM."""
            ps = projp.tile([128, 512], f32, tag="proj", name="oproj_ps")
            MM(
                ps[:],
                o_pair[:, T * TQB + i * 128 : T * TQB + (i + 1) * 128],
                wo_t[:],
                start=True,
                stop=True,
                skip_group_check=True,
            )
            ot = ostage.tile([128, 512], bf, tag="ot", name="ot")
            nc.vector.tensor_copy(out=ot[:], in_=ps[:])
            dma(out=out_p[T * TQB + i * 128 : T * TQB + (i + 1) * 128, :], in_=ot[:])

        def make_norm_steps(T, h, pvsb):
            """Closures normalizing (T, h)'s output from its SBUF copies."""
            steps = []
            for half in range(2):
                i = 4 * T + 2 * h + half

                def s1(i=i, pv1=pvsb[half]):
                    # bounce through DRAM to spread the denominator row over
                    # 64 partitions: DVE reciprocal is ~6.5 cycles per FREE
                    # element, so shape [64, 8] beats [1, 512] by ~15x
                    dma(out=rden[i : i + 1, :], in_=pv1[64:65, :])
                    sp = wspool.tile([64, 8], bf, tag="sp", name="sp")
                    dma(out=sp[:], in_=rden[i].rearrange("(p e) -> p e", p=64))
                    sp2 = wspool.tile([64, 8], f32, tag="sp2", name="sp2")
                    nc.vector.reciprocal(out=sp2[:], in_=sp[:])
                    dma(out=rrec[i].rearrange("(p e) -> p e", p=64), in_=sp2[:])

                def s2(i=i, half=half, h=h, T=T, pv1=pvsb[half]):
                    w = wspool.tile([64, 512], f32, tag="ws", name="wst")
                    dma(out=w[:], in_=rrec[i : i + 1, :].partition_broadcast(64))
                    nc.gpsimd.tensor_mul(
                        out=o_pair[
                            h * 64 : (h + 1) * 64,
                            T * TQB + half * 512 : T * TQB + half * 512 + 512,
                        ],
                        in0=pv1[0:64, :],
                        in1=w[:],
                    )

                steps.append(s1)
                steps.append(s2)
            return steps

        # ---- attention: flat unit stream, scores one chunk ahead ----
        def emit_scores(h, T, j):
            pb = h * 64
            sc = scorep.tile([128, 1024], f32, tag="sc", name="sc")
            for half in range(2):
                MM(
                    sc[:, half * 512 : half * 512 + 512],
                    kt[pb : pb + 64, j * 128 : (j + 1) * 128],
                    qt[pb : pb + 64, T * TQB + half * 512 : T * TQB + half * 512 + 512],
                    start=True,
                    stop=True,
                    skip_group_check=True,
                )
            return sc

        # prologue: kick all first-need DMAs in parallel, warm the PE on
        # dummy data while they land, then project K chunk 0 + Q chunks 0-1
        rk0 = load_raw(kT, 0, eng=nc.scalar)
        rq0 = load_raw(qT, 0)
        rq1 = load_raw(qT, 1, eng=nc.scalar)
        load_wv()
        load_vraw(0)
        for _ in range(12):
            wps = projp.tile([128, 512], f32, tag="proj", name="warm_ps")
            MM(
                wps[:],
                dummy[0:64, 0:128],
                dummy[0:64, :],
                start=True,
                stop=True,
                skip_group_check=True,
            )
        proj_from_raw(rk0, wk_t, bk_t, kt, 0)
        proj_from_raw(rq0, wq_t, bq_t, qt, 0)
        # first unit's scores half-a as soon as K0+Q0 exist; half-b after Q1.
        # Its exp is split into halves in the loop so half-a isn't gated on Q1.
        sc0 = scorep.tile([128, 1024], f32, tag="sc", name="sc")
        MM(
            sc0[:, 0:512],
            kt[0:64, 0:128],
            qt[0:64, 0:512],
            start=True,
            stop=True,
            skip_group_check=True,
        )
        proj_from_raw(rq1, wq_t, bq_t, qt, 1)
        MM(
            sc0[:, 512:1024],
            kt[0:64, 0:128],
            qt[0:64, 512:1024],
            start=True,
            stop=True,
            skip_group_check=True,
        )
        emit_v(0)

        # per-unit extra-work schedule: (h, T, j) -> list of closures
        extra = {}
        # segment (0,0): K chunks 1-7 (chunk c needed by scores j=4c, emitted
        # ~2 ahead), Q chunks 2-7 (needed from T=1 on)
        for n, jj in enumerate((2, 6, 10, 14, 18, 22, 26)):
            extra.setdefault((0, 0, jj), []).append(
                lambda c=n + 1: proj_chunk(kT, wk_t, bk_t, kt, c)
            )

        for n, jj in enumerate((3, 7, 11, 15, 19, 23)):
            extra.setdefault((1, 0, jj), []).append(
                lambda c=n + 2: proj_chunk(qT, wq_t, bq_t, qt, c)
            )
        extra[(1, 0, 1)] = [load_wo]

        # Last block (T=3): head-0 half of the out-projection runs inside the
        # (h=1) unit stream (slice i's columns are normalized by then); the
        # result is staged in SBUF so the tail only needs the head-1 half.
        pa_sb_tiles = []

        def emit_pa(i):
            col = (NT - 1) * TQB + i * 128
            ps = projp.tile([128, 512], f32, tag="proj", name="pa_ps")
            MM(
                ps[:],
                o_pair[0:64, col : col + 128],
                wo_t[0:64, :],
                start=True,
                stop=True,
                skip_group_check=True,
            )
            t = pasb.tile([128, 512], f32, tag="pasb", name="pa_sb")
            pa_sb_tiles.append(t)
            nc.vector.tensor_copy(out=t[:], in_=ps[:])

        for n, jj in enumerate((12, 14, 18, 20, 24, 26, 28, 30)):
            extra.setdefault((1, NT - 1, jj), []).append(lambda i=n: emit_pa(i))

        pend_norm = {}  # (T, h) -> norm step closures, filled as units finish
        norm_slots = {4: 0, 10: 1, 16: 2, 22: 3}
        oproj_slots = (2, 6, 9, 13, 17, 21, 25, 29)

        units = [(h, T, j) for T in range(NT) for h in range(HP) for j in range(32)]
        pv_tiles = {}
        pvsb_tiles = {}

        sc_next = sc0
        for idx, (h, T, j) in enumerate(units):
            if j == 0:
                pv_tiles[(T, h)] = [
                    pvp.tile([DK + 1, 512], f32, tag="pv", name=f"pv{_h}")
                    for _h in range(2)
                ]
            sc = sc_next
            pt = ppool.tile([128, 1024], bf, tag="pt", name="pt")
            if idx == 0:
                for hf in range(2):
                    nc.scalar.activation(
                        out=pt[:, hf * 512 : hf * 512 + 512],
                        in_=sc[:, hf * 512 : hf * 512 + 512],
                        func=Exp,
                        scale=0.125,
                    )
            else:
                nc.scalar.activation(out=pt[:], in_=sc[:], func=Exp, scale=0.125)
            # next unit's scores go to PE first so ACT(u+1) is never gated
            # by interleaved projection work
            if idx + 1 < len(units):
                nh, nT, nj = units[idx + 1]
                sc_next = emit_scores(nh, nT, nj)
            # interleaved extra work (runs on PE/DVE/DMA while ACT is busy)
            if h == 0 and T == 0 and j + 1 < 32:
                emit_v(j + 1)
            for fn in extra.get((h, T, j), ()):
                fn()
            # normalization of the previous (T, h) segment
            prev = (T, h - 1) if h > 0 else (T - 1, HP - 1)
            if j in norm_slots and prev in pend_norm:
                pend_norm[prev][norm_slots[j]]()
            # output projection of block T-1 during (T, h=1)
            if h == 1 and T >= 1 and j in oproj_slots:
                emit_oproj(T - 1, oproj_slots.index(j))
            pv = pv_tiles[(T, h)]
            for half in range(2):
                MM(
                    pv[half][:],
                    v_store[j][:, h, :],
                    pt[:, half * 512 : half * 512 + 512],
                    start=(j == 0),
                    stop=(j == 31),
                    skip_group_check=True,
                )
            if j == 31:
                pvsb = []
                for half in range(2):
                    t = wspool.tile([DK + 1, 512], bf, tag="pvsb", name="pvsb")
                    nc.vector.tensor_copy(out=t[:], in_=pv[half][:])
                    pvsb.append(t)
                pvsb_tiles[(T, h)] = pvsb
                if not (T == NT - 1 and h == HP - 1):
                    pend_norm[(T, h)] = make_norm_steps(T, h, pvsb)

        # Tail: block T=3. Head 0's half already ran inside (3,1)'s stream
        # (pa_sb tiles). Head 1 skips normalization: out uses the
        # UNNORMALIZED O1 (bf16 copy of pvsb) and the reciprocal is applied
        # per-partition after the out-projection.
        Tl = NT - 1
        pvsb1 = pvsb_tiles[(Tl, 1)]
        ps_rc = projp.tile([128, 512], f32, tag="proj", name="ps_rc")
        for i in range(8):
            MM(
                ps_rc[:, i : i + 1],
                pvsb1[i // 4][64:65, (i % 4) * 128 : (i % 4) * 128 + 128],
                ones_bf[64:65, :],
                start=True,
                stop=True,
                skip_group_check=True,
            )
        rc_all = wspool.tile([128, 8], f32, tag="rcall2", name="rc_all")
        nc.vector.reciprocal(out=rc_all[:], in_=ps_rc[:, 0:8])

        for i in range(8):
            col = Tl * TQB + i * 128
            pb = projp.tile([128, 512], f32, tag="proj", name="out_pb")
            MM(
                pb[:],
                pvsb1[i // 4][0:64, (i % 4) * 128 : (i % 4) * 128 + 128],
                wo_lo[:],
                start=True,
                stop=True,
                skip_group_check=True,
            )
            ot = ostage.tile([128, 512], bf, tag="ot", name="ot")
            nc.vector.scalar_tensor_tensor(
                out=ot[:],
                in0=pb[:],
                scalar=rc_all[:, i : i + 1],
                in1=pa_sb_tiles[i][:],
                op0=Mult,
                op1=Add,
            )
            dma(out=out_p[col : col + 128, :], in_=ot[:])

    if not nc.is_finalized():
        nc.finalize()
    return nc


def _get_program():
    global _PROGRAM
    if _PROGRAM is None:
        _PROGRAM = _build_program()
    return _PROGRAM


def _prep_inputs(q, k, v, w_q, b_q, w_k, b_k, w_v, b_v, w_o, b_o):
    bf16 = ml_dtypes.bfloat16
    q = np.asarray(q, dtype=np.float32)
    k = np.asarray(k, dtype=np.float32)
    v = np.asarray(v, dtype=np.float32)
    qT = [np.ascontiguousarray(q[b].T).astype(bf16) for b in range(B)]  # [D, S]
    kT = [np.ascontiguousarray(k[b].T).astype(bf16) for b in range(B)]
    vT = [np.ascontiguousarray(v[b].T).astype(bf16) for b in range(B)]
    w_q = np.asarray(w_q, np.float32)
    w_k = np.asarray(w_k, np.float32)
    w_v = np.asarray(w_v, np.float32)
    w_o = np.asarray(w_o, np.float32)

    in_maps = []
    for c in range(N_CORES):
        b, p = divmod(c, 4)
        P = slice(128 * p, 128 * p + 128)
        in_maps.append(
            {
                "qT": qT[b],
                "kT": kT[b],
                "vT": vT[b],
                "wqPT": np.ascontiguousarray(w_q[P, :].T).astype(bf16),
                "wkPT": np.ascontiguousarray(w_k[P, :].T).astype(bf16),
                "wvPT": np.ascontiguousarray(w_v[P, :].T).astype(bf16),
                "woP": np.ascontiguousarray(w_o[:, P].T).astype(bf16),
                "bqPc": np.ascontiguousarray(
                    np.asarray(b_q, np.float32)[P].reshape(128, 1)
                ),
                "bkPc": np.ascontiguousarray(
                    np.asarray(b_k, np.float32)[P].reshape(128, 1)
                ),
            }
        )
    return in_maps


def run_cores(in_maps, trace=False, **kw):
    """Compile+run the SPMD program; returns BassKernelResults."""
    from concourse.bass_utils import run_bass_kernel_spmd

    nc = _get_program()
    return run_bass_kernel_spmd(nc, in_maps, list(range(N_CORES)), trace=trace, **kw)


def _combine(results, b_v, b_o, w_o):
    bo_eff = (
        np.asarray(b_o, np.float32)
        + np.asarray(b_v, np.float32) @ np.asarray(w_o, np.float32).T
    )
    out = np.empty((B, S, D), np.float32)
    for b in range(B):
        acc = results[4 * b]["out"].astype(np.float32)
        for p in range(1, 4):
            acc += results[4 * b + p]["out"].astype(np.float32)
        out[b] = acc + bo_eff
    return out


def kernel(q, k, v, w_q, b_q, w_k, b_k, w_v, b_v, w_o, b_o):
    in_maps = _prep_inputs(q, k, v, w_q, b_q, w_k, b_k, w_v, b_v, w_o, b_o)
    res = run_cores(in_maps)
    return _combine(res.results, b_v, b_o, w_o)
